# revision 3
# baseline (speedup 1.0000x reference)
"""Distributed multi-head attention kernel for 8 TRN2 NeuronCores.

Reference computation (per batch b):
    qw/kw/vw = x @ W  (per-head slices of 64)
    a = softmax(qw @ kw^T / 8 - (1-v_mask)*1e10 - causal*1e10)
    out = (a @ vw) * q_mask

Sharding: core c handles batch b = c//2 and head-group hg = c%2 (4 of 8
heads = 256 output columns).  Each core's output slice is disjoint, so no
collectives are needed; the host concatenates the 8 slices.

Device algorithm (per core), all matmuls in float32r (TF32-like, full rate):
  - inputs are host-transposed to [D, S] so the contraction dim sits on
    SBUF partitions
  - projections: QW^T/KW^T [256, S] and VW [S, 256] (+ a ones column per
    head for the softmax row-sums)
  - scores in S^T layout [k, q]: per (head, k-chunk of 128), matmul against
    QW^T in q-tiles of 512; U = exp(0.125*scores + key_bias) via ACT with a
    per-partition bias (-1e10 for masked keys -> exp gives exactly 0)
  - causal masking: blocks strictly above the diagonal are never computed;
    the single ragged diagonal block per chunk is multiplied by a
    host-precomputed 0/1 pattern
  - PV: O^T[65, q] accumulated in PSUM over k-chunks; row 64 (ones column)
    is the softmax denominator
  - dead queries (rows whose causally-allowed keys are all masked would be
    0/0): the reference's fp32 rounding makes them a uniform average over
    "singly-masked" keys; host passes indicator columns and tiny N=4
    matmuls add that average into the first 4 output columns exactly
  - finalize: PE-transpose O^T -> [q, 65], scale by q_mask/rowsum, one 2MB
    output DMA
"""

import numpy as np

B, S, D = 4, 2048, 512
HG = 256          # output columns per core (4 heads x 64)
KS = 65           # head value width + ones column
NCH = 16          # k chunks of 128
NEG = np.float32(-1e10)

_CACHE = {}


def _build():
    import concourse.bass as bass  # noqa: F401
    from concourse import bacc
    import concourse.mybir as mybir
    from concourse.tile import TileContext

    F32 = mybir.dt.float32
    F32R = mybir.dt.float32r
    Exp = mybir.ActivationFunctionType.Exp

    nc = bacc.Bacc()
    qT = nc.declare_dram_parameter("qT", [D, S], F32R, isOutput=False)
    kT = nc.declare_dram_parameter("kT", [D, S], F32R, isOutput=False)
    vT = nc.declare_dram_parameter("vT", [D, S], F32R, isOutput=False)
    wq = nc.declare_dram_parameter("wq", [D, HG], F32R, isOutput=False)
    wk = nc.declare_dram_parameter("wk", [D, HG], F32R, isOutput=False)
    wv = nc.declare_dram_parameter("wv", [D, HG], F32R, isOutput=False)
    vbias = nc.declare_dram_parameter("vbias", [128, NCH], F32, isOutput=False)
    qmask = nc.declare_dram_parameter("qmask", [128, NCH], F32, isOutput=False)
    caus = nc.declare_dram_parameter("caus", [128, 4 * 512], F32R, isOutput=False)
    fixv = nc.declare_dram_parameter("fixv", [128, 4 * NCH], F32R, isOutput=False)
    ident = nc.declare_dram_parameter("ident", [128, 128], F32, isOutput=False)
    ones4 = nc.declare_dram_parameter("ones4", [128, 4], F32R, isOutput=False)
    out = nc.declare_dram_parameter("out", [S, HG], F32, isOutput=True)

    with TileContext(nc) as tc:
        with tc.tile_pool(name="sb", bufs=1) as sb, \
             tc.tile_pool(name="ps", bufs=1, space="PSUM") as ps:

            def sbt(name, shape, dtype, bufs=1, tag=None):
                return sb.tile(shape, dtype, name=name, tag=tag or name, bufs=bufs)

            vbias_sb = sbt("vbias_sb", [128, NCH], F32)
            nc.sync.dma_start(out=vbias_sb, in_=vbias[:])
            qmask_sb = sbt("qmask_sb", [128, NCH], F32)
            nc.sync.dma_start(out=qmask_sb, in_=qmask[:])
            caus_sb = sbt("caus_sb", [128, 4 * 512], F32R)
            nc.sync.dma_start(out=caus_sb, in_=caus[:])
            fixv_sb = sbt("fixv_sb", [128, 4 * NCH], F32R)
            nc.sync.dma_start(out=fixv_sb, in_=fixv[:])
            ident_sb = sbt("ident_sb", [128, 128], F32)
            nc.sync.dma_start(out=ident_sb, in_=ident[:])
            ones4_sb = sbt("ones4_sb", [128, 4], F32R)
            nc.sync.dma_start(out=ones4_sb, in_=ones4[:])

            w_sb = {}
            for nm, dram in (("q", wq), ("k", wk), ("v", wv)):
                for Dc in range(4):
                    t = sbt(f"w{nm}{Dc}", [128, HG], F32R)
                    nc.sync.dma_start(out=t, in_=dram[128 * Dc:128 * (Dc + 1), :])
                    w_sb[(nm, Dc)] = t

            qwT = [sbt(f"qwT{i}", [128, S], F32R) for i in range(2)]
            kwT = [sbt(f"kwT{i}", [128, S], F32R) for i in range(2)]
            vw = [sbt(f"vw{i}", [128, 4 * KS], F32R) for i in range(NCH)]

            def load_xT(dram):
                tiles = []
                for Dc in range(4):
                    t = sb.tile([128, S], F32R, name=f"xT{Dc}", tag=f"xT{Dc}", bufs=2)
                    nc.sync.dma_start(out=t, in_=dram[128 * Dc:128 * (Dc + 1), :])
                    tiles.append(t)
                return tiles

            # ---- projections ----
            vt = load_xT(vT)
            for st in range(NCH):
                p = ps.tile([128, HG], F32, name="pprj", tag="psS", bufs=2)
                for Dc in range(4):
                    nc.tensor.matmul(p, vt[Dc][:, 128 * st:128 * (st + 1)],
                                     w_sb[("v", Dc)], start=(Dc == 0), stop=(Dc == 3))
                t = vw[st]
                nc.vector.tensor_copy(
                    t.rearrange("p (h j) -> p h j", j=KS)[:, :, 64:65],
                    ones4_sb.rearrange("p (h o) -> p h o", o=1))
                nc.vector.tensor_copy(
                    t.rearrange("p (h j) -> p h j", j=KS)[:, :, 0:64],
                    p.rearrange("p (h j) -> p h j", j=64))

            for src, dst, wnm in ((kT, kwT, "k"), (qT, qwT, "q")):
                xt = load_xT(src)
                for dc in range(2):
                    for st2 in range(4):
                        p = ps.tile([128, 512], F32, name="pprj2", tag="psS", bufs=2)
                        for Dc in range(4):
                            nc.tensor.matmul(
                                p, w_sb[(wnm, Dc)][:, 128 * dc:128 * (dc + 1)],
                                xt[Dc][:, 512 * st2:512 * (st2 + 1)],
                                start=(Dc == 0), stop=(Dc == 3))
                        nc.scalar.copy(dst[dc][:, 512 * st2:512 * (st2 + 1)], p)

            # ---- attention per head ----
            ofin = sbt("ofin", [128, NCH * HG], F32)
            for h in range(4):
                dc, ho = h // 2, (h % 2) * 64
                kw_t, qw_t = kwT[dc], qwT[dc]
                psO = [ps.tile([KS, 512], F32, name=f"psO{t}", tag="psO", bufs=4)
                       for t in range(4)]
                for c in range(NCH):
                    t0 = c // 4
                    groups = []
                    for glo, ghi in ((0, 1), (2, 3)):
                        tiles = [t for t in (glo, ghi) if t >= t0]
                        if tiles:
                            groups.append(tiles)
                    for tiles in groups:
                        gn = len(tiles)
                        psS = ps.tile([128, 1024], F32, name="psS", tag="psS", bufs=2)
                        for i, t in enumerate(tiles):
                            nc.tensor.matmul(
                                psS[:, 512 * i:512 * (i + 1)],
                                kw_t[ho:ho + 64, 128 * c:128 * (c + 1)],
                                qw_t[ho:ho + 64, 512 * t:512 * (t + 1)],
                                start=True, stop=True)
                        U = sb.tile([128, 1024], F32R, name="U", tag="U", bufs=3)
                        nc.scalar.activation(U[:, :512 * gn], psS[:, :512 * gn], Exp,
                                             bias=vbias_sb[:, c:c + 1], scale=0.125)
                        if tiles[0] == t0:
                            g = c % 4
                            nc.vector.tensor_mul(U[:, 0:512], U[:, 0:512],
                                                 caus_sb[:, 512 * g:512 * (g + 1)])
                        for i, t in enumerate(tiles):
                            stop = (c == 4 * t + 3) if t > 0 else False
                            nc.tensor.matmul(psO[t], vw[c][:, KS * h:KS * (h + 1)],
                                             U[:, 512 * i:512 * (i + 1)],
                                             start=(c == 0), stop=stop,
                                             skip_group_check=True)
                    nc.tensor.matmul(psO[0][:, 0:4], vw[c][:, KS * h:KS * (h + 1)],
                                     fixv_sb[:, 4 * c:4 * (c + 1)],
                                     start=False, stop=(c == NCH - 1),
                                     skip_group_check=True)
                # finalize
                for t in range(4):
                    ot = sb.tile([KS, 512], F32, name="ot", tag="ot", bufs=2)
                    nc.scalar.copy(ot, psO[t])
                    tp = ps.tile([128, 4 * KS], F32, name="tp", tag="psS", bufs=2)
                    for j in range(4):
                        nc.tensor.matmul(tp[:, KS * j:KS * j + KS],
                                         ot[:, 128 * j:128 * (j + 1)],
                                         ident_sb[0:KS, 0:KS],
                                         is_transpose=True,
                                         start=(j == 0), stop=(j == 3),
                                         skip_group_check=True)
                    rs = sb.tile([128, 4], F32, name="rs", tag="rs", bufs=2)
                    nc.vector.tensor_scalar_add(
                        rs.rearrange("p (j o) -> p j o", o=1),
                        tp.rearrange("p (j f) -> p j f", f=KS)[:, :, 64:65], 1e-30)
                    rcp = sb.tile([128, 4], F32, name="rcp", tag="rcp", bufs=2)
                    nc.vector.reciprocal(rcp, rs)
                    scl = sb.tile([128, 4], F32, name="scl", tag="scl", bufs=2)
                    nc.vector.tensor_mul(scl, rcp, qmask_sb[:, 4 * t:4 * (t + 1)])
                    for j in range(4):
                        col = (4 * t + j) * HG + 64 * h
                        nc.vector.tensor_scalar_mul(
                            ofin[:, col:col + 64], tp[:, KS * j:KS * j + 64],
                            scl[:, j:j + 1])
            nc.sync.dma_start(out=out.rearrange("(j p) n -> p j n", p=128),
                              in_=ofin.rearrange("p (j n) -> p j n", n=HG))

    nc.compile()
    return nc


def _prep_inputs(q, k, v, v_mask, q_mask, Wq, Wk, Wv):
    q = np.asarray(q, np.float32)
    k = np.asarray(k, np.float32)
    v = np.asarray(v, np.float32)
    v_mask = np.asarray(v_mask, np.float32)
    q_mask = np.asarray(q_mask, np.float32)
    Wq = np.asarray(Wq, np.float32)
    Wk = np.asarray(Wk, np.float32)
    Wv = np.asarray(Wv, np.float32)

    p = np.arange(128)[:, None]
    j = np.arange(512)[None, :]
    causp = np.concatenate(
        [(p + 128 * g <= j).astype(np.float32) for g in range(4)], axis=1)
    ident = np.eye(128, dtype=np.float32)

    in_maps = []
    for core in range(8):
        b, hg = core // 2, core % 2
        cs = slice(hg * HG, (hg + 1) * HG)
        vb = (NEG * (1.0 - v_mask[b])).astype(np.float32)
        fix = np.zeros((S, 4), np.float32)
        if v_mask[b, 0] == 0:
            first_one = int(np.argmax(v_mask[b] > 0))
            ks = np.arange(S)
            for dj in range(min(first_one, 4)):
                sel = ((ks <= dj) & (v_mask[b] == 0)) | ((ks > dj) & (v_mask[b] == 1))
                fix[:, dj] = sel.astype(np.float32)
        in_maps.append({
            "qT": np.ascontiguousarray(q[b].T),
            "kT": np.ascontiguousarray(k[b].T),
            "vT": np.ascontiguousarray(v[b].T),
            "wq": np.ascontiguousarray(Wq[:, cs]),
            "wk": np.ascontiguousarray(Wk[:, cs]),
            "wv": np.ascontiguousarray(Wv[:, cs]),
            "vbias": np.ascontiguousarray(vb.reshape(NCH, 128).T),
            "qmask": np.ascontiguousarray(q_mask[b].reshape(NCH, 128).T),
            "caus": causp,
            "fixv": np.ascontiguousarray(
                fix.reshape(NCH, 128, 4).transpose(1, 0, 2).reshape(128, 4 * NCH)),
            "ident": ident,
            "ones4": np.ones((128, 4), np.float32),
        })
    return in_maps


def kernel(q, k, v, v_mask, q_mask, Wq, Wk, Wv, _trace=False):
    from concourse.bass_utils import run_bass_kernel_spmd

    if "nc" not in _CACHE:
        _CACHE["nc"] = _build()
    nc = _CACHE["nc"]
    in_maps = _prep_inputs(q, k, v, v_mask, q_mask, Wq, Wk, Wv)
    res = run_bass_kernel_spmd(nc, in_maps, core_ids=list(range(8)), trace=_trace)
    _CACHE["last_result"] = res
    full = np.zeros((B, S, 2 * HG), np.float32)
    for core in range(8):
        b, hg = core // 2, core % 2
        full[b, :, hg * HG:(hg + 1) * HG] = res.results[core]["out"]
    return full


# revision 4
# speedup vs baseline: 1.2161x; 1.2161x over previous
"""Distributed multi-head attention kernel for 8 TRN2 NeuronCores.

Reference computation (per batch b):
    qw/kw/vw = x @ W  (per-head slices of 64)
    a = softmax(qw @ kw^T / 8 - (1-v_mask)*1e10 - causal*1e10)
    out = (a @ vw) * q_mask

Sharding: core c handles batch b = c//2 and head-group hg = c%2 (4 of 8
heads = 256 output columns).  Each core's output slice is disjoint, so no
collectives are needed; the host concatenates the 8 slices.

Device algorithm (per core), all matmuls in float32r (TF32-like, full rate):
  - inputs are host-transposed to [D, S] so the contraction dim sits on
    SBUF partitions
  - projections: QW^T/KW^T [256, S] and VW [S, 256] (+ a ones column per
    head for the softmax row-sums)
  - scores in S^T layout [k, q]: per (head, k-chunk of 128), matmul against
    QW^T in q-tiles of 512; U = exp(0.125*scores + key_bias) via ACT with a
    per-partition bias (-1e10 for masked keys -> exp gives exactly 0)
  - causal masking: blocks strictly above the diagonal are never computed;
    the single ragged diagonal block per chunk is multiplied by a
    host-precomputed 0/1 pattern
  - PV: O^T[65, q] accumulated in PSUM over k-chunks; row 64 (ones column)
    is the softmax denominator
  - dead queries (rows whose causally-allowed keys are all masked would be
    0/0): the reference's fp32 rounding makes them a uniform average over
    "singly-masked" keys; host passes indicator columns and tiny N=4
    matmuls add that average into the first 4 output columns exactly
  - finalize: PE-transpose O^T -> [q, 65], scale by q_mask/rowsum, one 2MB
    output DMA
"""

import numpy as np
import ml_dtypes

BF = ml_dtypes.bfloat16
B, S, D = 4, 2048, 512
HG = 256          # output columns per core (4 heads x 64)
KS = 65           # head value width + ones column
NCH = 16          # k chunks of 128
NEG = np.float32(-1e10)

_CACHE = {}


def _build():
    import concourse.bass as bass  # noqa: F401
    from concourse import bacc
    import concourse.mybir as mybir
    from concourse.tile import TileContext

    F32 = mybir.dt.float32
    BF16 = mybir.dt.bfloat16
    Exp = mybir.ActivationFunctionType.Exp

    nc = bacc.Bacc()
    qT = nc.declare_dram_parameter("qT", [D, S], BF16, isOutput=False)
    kT = nc.declare_dram_parameter("kT", [D, S], BF16, isOutput=False)
    vT = nc.declare_dram_parameter("vT", [D, S], BF16, isOutput=False)
    wq = nc.declare_dram_parameter("wq", [D, HG], BF16, isOutput=False)
    wk = nc.declare_dram_parameter("wk", [D, HG], BF16, isOutput=False)
    wv = nc.declare_dram_parameter("wv", [D, HG], BF16, isOutput=False)
    vbias = nc.declare_dram_parameter("vbias", [128, NCH], F32, isOutput=False)
    qmask = nc.declare_dram_parameter("qmask", [128, NCH], F32, isOutput=False)
    caus = nc.declare_dram_parameter("caus", [128, 4 * 512], BF16, isOutput=False)
    fixv = nc.declare_dram_parameter("fixv", [128, 4 * NCH], BF16, isOutput=False)
    ident = nc.declare_dram_parameter("ident", [128, 128], F32, isOutput=False)
    ones4 = nc.declare_dram_parameter("ones4", [128, 4], BF16, isOutput=False)
    out = nc.declare_dram_parameter("out", [S, HG], F32, isOutput=True)

    with TileContext(nc) as tc:
        with tc.tile_pool(name="sb", bufs=1) as sb, \
             tc.tile_pool(name="ps", bufs=1, space="PSUM") as ps:

            def sbt(name, shape, dtype, bufs=1, tag=None):
                return sb.tile(shape, dtype, name=name, tag=tag or name, bufs=bufs)

            vbias_sb = sbt("vbias_sb", [128, NCH], F32)
            nc.sync.dma_start(out=vbias_sb, in_=vbias[:])
            qmask_sb = sbt("qmask_sb", [128, NCH], F32)
            nc.sync.dma_start(out=qmask_sb, in_=qmask[:])
            caus_sb = sbt("caus_sb", [128, 4 * 512], BF16)
            nc.sync.dma_start(out=caus_sb, in_=caus[:])
            fixv_sb = sbt("fixv_sb", [128, 4 * NCH], BF16)
            nc.sync.dma_start(out=fixv_sb, in_=fixv[:])
            ident_sb = sbt("ident_sb", [128, 128], F32)
            nc.sync.dma_start(out=ident_sb, in_=ident[:])
            ones4_sb = sbt("ones4_sb", [128, 4], BF16)
            nc.sync.dma_start(out=ones4_sb, in_=ones4[:])

            w_sb = {}
            for nm, dram in (("q", wq), ("k", wk), ("v", wv)):
                for Dc in range(4):
                    t = sbt(f"w{nm}{Dc}", [128, HG], BF16)
                    nc.sync.dma_start(out=t, in_=dram[128 * Dc:128 * (Dc + 1), :])
                    w_sb[(nm, Dc)] = t

            qwT = [sbt(f"qwT{i}", [128, S], BF16) for i in range(2)]
            kwT = [sbt(f"kwT{i}", [128, S], BF16) for i in range(2)]
            vw = [sbt(f"vw{i}", [128, 4 * KS], BF16) for i in range(NCH)]

            def load_xT(dram):
                tiles = []
                for Dc in range(4):
                    t = sb.tile([128, S], BF16, name=f"xT{Dc}", tag=f"xT{Dc}", bufs=2)
                    nc.sync.dma_start(out=t, in_=dram[128 * Dc:128 * (Dc + 1), :])
                    tiles.append(t)
                return tiles

            # ---- projections ----
            vt = load_xT(vT)
            for st in range(NCH):
                p = ps.tile([128, HG], F32, name="pprj", tag="psS", bufs=2)
                for Dc in range(4):
                    nc.tensor.matmul(p, vt[Dc][:, 128 * st:128 * (st + 1)],
                                     w_sb[("v", Dc)], start=(Dc == 0), stop=(Dc == 3))
                t = vw[st]
                nc.vector.tensor_copy(
                    t.rearrange("p (h j) -> p h j", j=KS)[:, :, 64:65],
                    ones4_sb.rearrange("p (h o) -> p h o", o=1))
                nc.vector.tensor_copy(
                    t.rearrange("p (h j) -> p h j", j=KS)[:, :, 0:64],
                    p.rearrange("p (h j) -> p h j", j=64))

            for src, dst, wnm in ((kT, kwT, "k"), (qT, qwT, "q")):
                xt = load_xT(src)
                for dc in range(2):
                    for st2 in range(4):
                        p = ps.tile([128, 512], F32, name="pprj2", tag="psS", bufs=2)
                        for Dc in range(4):
                            nc.tensor.matmul(
                                p, w_sb[(wnm, Dc)][:, 128 * dc:128 * (dc + 1)],
                                xt[Dc][:, 512 * st2:512 * (st2 + 1)],
                                start=(Dc == 0), stop=(Dc == 3))
                        nc.scalar.copy(dst[dc][:, 512 * st2:512 * (st2 + 1)], p)

            # ---- attention per head ----
            ofin = sbt("ofin", [128, NCH * HG], F32)
            for h in range(4):
                dc, ho = h // 2, (h % 2) * 64
                kw_t, qw_t = kwT[dc], qwT[dc]
                psO = [ps.tile([KS, 512], F32, name=f"psO{t}", tag="psO", bufs=4)
                       for t in range(4)]
                for c in range(NCH):
                    t0 = c // 4
                    groups = []
                    for glo, ghi in ((0, 1), (2, 3)):
                        tiles = [t for t in (glo, ghi) if t >= t0]
                        if tiles:
                            groups.append(tiles)
                    for tiles in groups:
                        gn = len(tiles)
                        psS = ps.tile([128, 1024], F32, name="psS", tag="psS", bufs=2)
                        for i, t in enumerate(tiles):
                            nc.tensor.matmul(
                                psS[:, 512 * i:512 * (i + 1)],
                                kw_t[ho:ho + 64, 128 * c:128 * (c + 1)],
                                qw_t[ho:ho + 64, 512 * t:512 * (t + 1)],
                                start=True, stop=True)
                        U = sb.tile([128, 1024], BF16, name="U", tag="U", bufs=3)
                        nc.scalar.activation(U[:, :512 * gn], psS[:, :512 * gn], Exp,
                                             bias=vbias_sb[:, c:c + 1], scale=0.125)
                        if tiles[0] == t0:
                            g = c % 4
                            nc.vector.tensor_mul(U[:, 0:512], U[:, 0:512],
                                                 caus_sb[:, 512 * g:512 * (g + 1)])
                        for i, t in enumerate(tiles):
                            stop = (c == 4 * t + 3) if t > 0 else False
                            nc.tensor.matmul(psO[t], vw[c][:, KS * h:KS * (h + 1)],
                                             U[:, 512 * i:512 * (i + 1)],
                                             start=(c == 0), stop=stop,
                                             skip_group_check=True)
                    nc.tensor.matmul(psO[0][:, 0:4], vw[c][:, KS * h:KS * (h + 1)],
                                     fixv_sb[:, 4 * c:4 * (c + 1)],
                                     start=False, stop=(c == NCH - 1),
                                     skip_group_check=True)
                # finalize
                for t in range(4):
                    ot = sb.tile([KS, 512], F32, name="ot", tag="ot", bufs=2)
                    nc.scalar.copy(ot, psO[t])
                    tp = ps.tile([128, 4 * KS], F32, name="tp", tag="psS", bufs=2)
                    for j in range(4):
                        nc.tensor.matmul(tp[:, KS * j:KS * j + KS],
                                         ot[:, 128 * j:128 * (j + 1)],
                                         ident_sb[0:KS, 0:KS],
                                         is_transpose=True,
                                         start=(j == 0), stop=(j == 3),
                                         skip_group_check=True)
                    rs = sb.tile([128, 4], F32, name="rs", tag="rs", bufs=2)
                    nc.vector.tensor_scalar_add(
                        rs.rearrange("p (j o) -> p j o", o=1),
                        tp.rearrange("p (j f) -> p j f", f=KS)[:, :, 64:65], 1e-30)
                    rcp = sb.tile([128, 4], F32, name="rcp", tag="rcp", bufs=2)
                    nc.vector.reciprocal(rcp, rs)
                    scl = sb.tile([128, 4], F32, name="scl", tag="scl", bufs=2)
                    nc.vector.tensor_mul(scl, rcp, qmask_sb[:, 4 * t:4 * (t + 1)])
                    for j in range(4):
                        col = (4 * t + j) * HG + 64 * h
                        nc.vector.tensor_scalar_mul(
                            ofin[:, col:col + 64], tp[:, KS * j:KS * j + 64],
                            scl[:, j:j + 1])
            nc.sync.dma_start(out=out.rearrange("(j p) n -> p j n", p=128),
                              in_=ofin.rearrange("p (j n) -> p j n", n=HG))

    nc.compile()
    return nc


def _prep_inputs(q, k, v, v_mask, q_mask, Wq, Wk, Wv):
    q = np.asarray(q, np.float32)
    k = np.asarray(k, np.float32)
    v = np.asarray(v, np.float32)
    v_mask = np.asarray(v_mask, np.float32)
    q_mask = np.asarray(q_mask, np.float32)
    Wq = np.asarray(Wq, np.float32)
    Wk = np.asarray(Wk, np.float32)
    Wv = np.asarray(Wv, np.float32)

    p = np.arange(128)[:, None]
    j = np.arange(512)[None, :]
    causp = np.concatenate(
        [(p + 128 * g <= j).astype(np.float32) for g in range(4)], axis=1)
    ident = np.eye(128, dtype=np.float32)

    in_maps = []
    for core in range(8):
        b, hg = core // 2, core % 2
        cs = slice(hg * HG, (hg + 1) * HG)
        vb = (NEG * (1.0 - v_mask[b])).astype(np.float32)
        fix = np.zeros((S, 4), np.float32)
        if v_mask[b, 0] == 0:
            first_one = int(np.argmax(v_mask[b] > 0))
            ks = np.arange(S)
            for dj in range(min(first_one, 4)):
                sel = ((ks <= dj) & (v_mask[b] == 0)) | ((ks > dj) & (v_mask[b] == 1))
                fix[:, dj] = sel.astype(np.float32)
        in_maps.append({
            "qT": np.ascontiguousarray(q[b].T).astype(BF),
            "kT": np.ascontiguousarray(k[b].T).astype(BF),
            "vT": np.ascontiguousarray(v[b].T).astype(BF),
            "wq": np.ascontiguousarray(Wq[:, cs]).astype(BF),
            "wk": np.ascontiguousarray(Wk[:, cs]).astype(BF),
            "wv": np.ascontiguousarray(Wv[:, cs]).astype(BF),
            "vbias": np.ascontiguousarray(vb.reshape(NCH, 128).T),
            "qmask": np.ascontiguousarray(q_mask[b].reshape(NCH, 128).T),
            "caus": causp.astype(BF),
            "fixv": np.ascontiguousarray(
                fix.reshape(NCH, 128, 4).transpose(1, 0, 2).reshape(128, 4 * NCH)).astype(BF),
            "ident": ident,
            "ones4": np.ones((128, 4), BF),
        })
    return in_maps


def kernel(q, k, v, v_mask, q_mask, Wq, Wk, Wv, _trace=False):
    from concourse.bass_utils import run_bass_kernel_spmd

    if "nc" not in _CACHE:
        _CACHE["nc"] = _build()
    nc = _CACHE["nc"]
    in_maps = _prep_inputs(q, k, v, v_mask, q_mask, Wq, Wk, Wv)
    res = run_bass_kernel_spmd(nc, in_maps, core_ids=list(range(8)), trace=_trace)
    _CACHE["last_result"] = res
    full = np.zeros((B, S, 2 * HG), np.float32)
    for core in range(8):
        b, hg = core // 2, core % 2
        full[b, :, hg * HG:(hg + 1) * HG] = res.results[core]["out"]
    return full


# revision 8
# speedup vs baseline: 1.5222x; 1.2517x over previous
"""Distributed multi-head attention kernel for 8 TRN2 NeuronCores.

Reference computation (per batch b):
    qw/kw/vw = x @ W  (per-head slices of 64)
    a = softmax(qw @ kw^T / 8 - (1-v_mask)*1e10 - causal*1e10)
    out = (a @ vw) * q_mask

Sharding: core c handles batch b = c//2 and head-group hg = c%2 (4 of 8
heads = 256 output columns).  Each core's output slice is disjoint, so no
collectives are needed; the host concatenates the 8 slices.

Device algorithm (per core), all matmuls in float32r (TF32-like, full rate):
  - inputs are host-transposed to [D, S] so the contraction dim sits on
    SBUF partitions
  - projections: QW^T/KW^T [256, S] and VW [S, 256] (+ a ones column per
    head for the softmax row-sums)
  - scores in S^T layout [k, q]: per (head, k-chunk of 128), matmul against
    QW^T in q-tiles of 512; U = exp(0.125*scores + key_bias) via ACT with a
    per-partition bias (-1e10 for masked keys -> exp gives exactly 0)
  - causal masking: blocks strictly above the diagonal are never computed;
    the single ragged diagonal block per chunk is multiplied by a
    host-precomputed 0/1 pattern
  - PV: O^T[65, q] accumulated in PSUM over k-chunks; row 64 (ones column)
    is the softmax denominator
  - dead queries (rows whose causally-allowed keys are all masked would be
    0/0): the reference's fp32 rounding makes them a uniform average over
    "singly-masked" keys; host passes indicator columns and tiny N=4
    matmuls add that average into the first 4 output columns exactly
  - finalize: PE-transpose O^T -> [q, 65], scale by q_mask/rowsum, one 2MB
    output DMA
"""

import numpy as np
import ml_dtypes

BF = ml_dtypes.bfloat16
B, S, D = 4, 2048, 512
HG = 256          # output columns per core (4 heads x 64)
KS = 65           # head value width + ones column
NCH = 16          # k chunks of 128
NEG = np.float32(-1e10)

_CACHE = {}


def _build():
    import concourse.bass as bass  # noqa: F401
    from concourse import bacc
    import concourse.mybir as mybir
    from concourse.tile import TileContext

    F32 = mybir.dt.float32
    BF16 = mybir.dt.bfloat16
    Exp = mybir.ActivationFunctionType.Exp

    nc = bacc.Bacc()
    qT = nc.declare_dram_parameter("qT", [D, S], BF16, isOutput=False)
    kT = nc.declare_dram_parameter("kT", [D, S], BF16, isOutput=False)
    vT = nc.declare_dram_parameter("vT", [D, S], BF16, isOutput=False)
    wq = nc.declare_dram_parameter("wq", [D, HG], BF16, isOutput=False)
    wk = nc.declare_dram_parameter("wk", [D, HG], BF16, isOutput=False)
    wv = nc.declare_dram_parameter("wv", [D, HG], BF16, isOutput=False)
    vbias = nc.declare_dram_parameter("vbias", [128, NCH], F32, isOutput=False)
    qmask = nc.declare_dram_parameter("qmask", [128, NCH], F32, isOutput=False)
    caus = nc.declare_dram_parameter("caus", [128, 4 * 512], BF16, isOutput=False)
    fixv = nc.declare_dram_parameter("fixv", [128, 4 * NCH], BF16, isOutput=False)
    ident = nc.declare_dram_parameter("ident", [128, 128], BF16, isOutput=False)
    ones4 = nc.declare_dram_parameter("ones4", [128, 4], BF16, isOutput=False)
    out = nc.declare_dram_parameter("out", [S, HG], F32, isOutput=True)

    with TileContext(nc) as tc:
        with tc.tile_pool(name="sb", bufs=1) as sb, \
             tc.tile_pool(name="ps", bufs=1, space="PSUM") as ps:

            def sbt(name, shape, dtype, bufs=1, tag=None):
                return sb.tile(shape, dtype, name=name, tag=tag or name, bufs=bufs)

            vbias_sb = sbt("vbias_sb", [128, NCH], F32)
            nc.sync.dma_start(out=vbias_sb, in_=vbias[:])
            qmask_sb = sbt("qmask_sb", [128, NCH], F32)
            nc.sync.dma_start(out=qmask_sb, in_=qmask[:])
            caus_sb = sbt("caus_sb", [128, 4 * 512], BF16)
            nc.sync.dma_start(out=caus_sb, in_=caus[:])
            fixv_sb = sbt("fixv_sb", [128, 4 * NCH], BF16)
            nc.sync.dma_start(out=fixv_sb, in_=fixv[:])
            ident_sb = sbt("ident_sb", [128, 128], BF16)
            nc.sync.dma_start(out=ident_sb, in_=ident[:])
            ones4_sb = sbt("ones4_sb", [128, 4], BF16)
            nc.sync.dma_start(out=ones4_sb, in_=ones4[:])

            w_sb = {}
            for nm, dram in (("q", wq), ("k", wk), ("v", wv)):
                for Dc in range(4):
                    t = sbt(f"w{nm}{Dc}", [128, HG], BF16)
                    nc.sync.dma_start(out=t, in_=dram[128 * Dc:128 * (Dc + 1), :])
                    w_sb[(nm, Dc)] = t

            qwT = [sbt(f"qwT{i}", [128, S], BF16) for i in range(2)]
            kwT = [sbt(f"kwT{i}", [128, S], BF16) for i in range(2)]
            vw = [sbt(f"vw{i}", [128, 4 * KS], BF16) for i in range(NCH)]

            def load_xT(dram):
                tiles = []
                for Dc in range(4):
                    t = sb.tile([128, S], BF16, name=f"xT{Dc}", tag=f"xT{Dc}", bufs=2)
                    nc.sync.dma_start(out=t, in_=dram[128 * Dc:128 * (Dc + 1), :])
                    tiles.append(t)
                return tiles

            # ---- projections ----
            vt = load_xT(vT)
            for st in range(NCH):
                p = ps.tile([128, HG], F32, name="pprj", tag="psS", bufs=3)
                for Dc in range(4):
                    nc.tensor.matmul(p, vt[Dc][:, 128 * st:128 * (st + 1)],
                                     w_sb[("v", Dc)], start=(Dc == 0), stop=(Dc == 3))
                t = vw[st]
                nc.vector.tensor_copy(
                    t.rearrange("p (h j) -> p h j", j=KS)[:, :, 64:65],
                    ones4_sb.rearrange("p (h o) -> p h o", o=1))
                nc.vector.tensor_copy(
                    t.rearrange("p (h j) -> p h j", j=KS)[:, :, 0:64],
                    p.rearrange("p (h j) -> p h j", j=64))

            for src, dst, wnm in ((kT, kwT, "k"), (qT, qwT, "q")):
                xt = load_xT(src)
                for dc in range(2):
                    for st2 in range(4):
                        p = ps.tile([128, 512], F32, name="pprj2", tag="psS", bufs=3)
                        for Dc in range(4):
                            nc.tensor.matmul(
                                p, w_sb[(wnm, Dc)][:, 128 * dc:128 * (dc + 1)],
                                xt[Dc][:, 512 * st2:512 * (st2 + 1)],
                                start=(Dc == 0), stop=(Dc == 3))
                        nc.scalar.copy(dst[dc][:, 512 * st2:512 * (st2 + 1)], p)

            # ---- attention: head pairs x q-tile passes ----
            # For head pair (2dc, 2dc+1) and q-tile t, iterate k-chunks c.
            # Both heads' score matmuls go to different PE row groups
            # (partition base 0 / 64) so they run concurrently; one [128,
            # 1024] PSUM tile holds both heads' scores and a single exp
            # covers both (the key bias is head-independent).
            ofin = sbt("ofin", [128, NCH * HG], F32)
            for dc in range(2):
                h0, h1 = 2 * dc, 2 * dc + 1
                kw_t, qw_t = kwT[dc], qwT[dc]
                for t in range(4):
                    psO = {}
                    for hh in (h0, h1):
                        psO[hh] = ps.tile([KS, 512], F32, name=f"psO{hh}",
                                          tag="psO", bufs=2)
                    cmax = 4 * t + 3
                    cend = NCH if t == 0 else cmax + 1
                    for c in range(cend):
                        if c <= cmax:
                            psS = ps.tile([128, 1024], F32, name="psS",
                                          tag="psS", bufs=3)
                            for i, (hh, ho) in enumerate(((h0, 0), (h1, 64))):
                                nc.tensor.matmul(
                                    psS[:, 512 * i:512 * (i + 1)],
                                    kw_t[ho:ho + 64, 128 * c:128 * (c + 1)],
                                    qw_t[ho:ho + 64, 512 * t:512 * (t + 1)],
                                    start=True, stop=True)
                            U = sb.tile([128, 1024], BF16, name="U", tag="U",
                                        bufs=4)
                            nc.scalar.activation(U, psS, Exp,
                                                 bias=vbias_sb[:, c:c + 1],
                                                 scale=0.125)
                            if c // 4 == t:
                                g = c % 4
                                for i in range(2):
                                    nc.vector.tensor_mul(
                                        U[:, 512 * i:512 * (i + 1)],
                                        U[:, 512 * i:512 * (i + 1)],
                                        caus_sb[:, 512 * g:512 * (g + 1)])
                            for i, hh in enumerate((h0, h1)):
                                stop = (c == cmax) if t > 0 else False
                                nc.tensor.matmul(psO[hh],
                                                 vw[c][:, KS * hh:KS * (hh + 1)],
                                                 U[:, 512 * i:512 * (i + 1)],
                                                 start=(c == 0), stop=stop,
                                                 skip_group_check=True)
                        if t == 0:
                            for hh in (h0, h1):
                                nc.tensor.matmul(
                                    psO[hh][:, 0:4],
                                    vw[c][:, KS * hh:KS * (hh + 1)],
                                    fixv_sb[:, 4 * c:4 * (c + 1)],
                                    start=False, stop=(c == NCH - 1),
                                    skip_group_check=True)
                    # finalize this q-tile for both heads
                    for hh in (h0, h1):
                        ot = sb.tile([KS, 512], BF16, name="ot", tag="ot", bufs=2)
                        nc.vector.tensor_copy(ot, psO[hh])
                        tp = ps.tile([128, 4 * 66], BF16, name="tp", tag="psS",
                                     bufs=3)
                        for j in range(4):
                            nc.tensor.matmul(tp[:, 66 * j:66 * j + KS],
                                             ot[:, 128 * j:128 * (j + 1)],
                                             ident_sb[0:KS, 0:KS],
                                             is_transpose=True,
                                             start=(j == 0), stop=(j == 3),
                                             skip_group_check=True)
                        rs = sb.tile([128, 4], F32, name="rs", tag="rs", bufs=2)
                        nc.vector.tensor_scalar_add(
                            rs.rearrange("p (j o) -> p j o", o=1),
                            tp.rearrange("p (j f) -> p j f", f=66)[:, :, 64:65],
                            1e-30)
                        rcp = sb.tile([128, 4], F32, name="rcp", tag="rcp", bufs=2)
                        nc.vector.reciprocal(rcp, rs)
                        scl = sb.tile([128, 4], F32, name="scl", tag="scl", bufs=2)
                        nc.vector.tensor_mul(scl, rcp, qmask_sb[:, 4 * t:4 * (t + 1)])
                        for j in range(4):
                            col = (4 * t + j) * HG + 64 * hh
                            nc.vector.tensor_scalar_mul(
                                ofin[:, col:col + 64], tp[:, 66 * j:66 * j + 64],
                                scl[:, j:j + 1])
            nc.sync.dma_start(out=out.rearrange("(j p) n -> p j n", p=128),
                              in_=ofin.rearrange("p (j n) -> p j n", n=HG))

    nc.compile()
    return nc


def _prep_inputs(q, k, v, v_mask, q_mask, Wq, Wk, Wv):
    q = np.asarray(q, np.float32)
    k = np.asarray(k, np.float32)
    v = np.asarray(v, np.float32)
    v_mask = np.asarray(v_mask, np.float32)
    q_mask = np.asarray(q_mask, np.float32)
    Wq = np.asarray(Wq, np.float32)
    Wk = np.asarray(Wk, np.float32)
    Wv = np.asarray(Wv, np.float32)

    p = np.arange(128)[:, None]
    j = np.arange(512)[None, :]
    causp = np.concatenate(
        [(p + 128 * g <= j).astype(np.float32) for g in range(4)], axis=1)
    ident = np.eye(128, dtype=np.float32)

    in_maps = []
    for core in range(8):
        b, hg = core // 2, core % 2
        cs = slice(hg * HG, (hg + 1) * HG)
        vb = (NEG * (1.0 - v_mask[b])).astype(np.float32)
        fix = np.zeros((S, 4), np.float32)
        if v_mask[b, 0] == 0:
            first_one = int(np.argmax(v_mask[b] > 0))
            ks = np.arange(S)
            for dj in range(min(first_one, 4)):
                sel = ((ks <= dj) & (v_mask[b] == 0)) | ((ks > dj) & (v_mask[b] == 1))
                fix[:, dj] = sel.astype(np.float32)
        in_maps.append({
            "qT": np.ascontiguousarray(q[b].T).astype(BF),
            "kT": np.ascontiguousarray(k[b].T).astype(BF),
            "vT": np.ascontiguousarray(v[b].T).astype(BF),
            "wq": np.ascontiguousarray(Wq[:, cs]).astype(BF),
            "wk": np.ascontiguousarray(Wk[:, cs]).astype(BF),
            "wv": np.ascontiguousarray(Wv[:, cs]).astype(BF),
            "vbias": np.ascontiguousarray(vb.reshape(NCH, 128).T),
            "qmask": np.ascontiguousarray(q_mask[b].reshape(NCH, 128).T),
            "caus": causp.astype(BF),
            "fixv": np.ascontiguousarray(
                fix.reshape(NCH, 128, 4).transpose(1, 0, 2).reshape(128, 4 * NCH)).astype(BF),
            "ident": ident.astype(BF),
            "ones4": np.ones((128, 4), BF),
        })
    return in_maps


def kernel(q, k, v, v_mask, q_mask, Wq, Wk, Wv, _trace=False):
    from concourse.bass_utils import run_bass_kernel_spmd

    if "nc" not in _CACHE:
        _CACHE["nc"] = _build()
    nc = _CACHE["nc"]
    in_maps = _prep_inputs(q, k, v, v_mask, q_mask, Wq, Wk, Wv)
    res = run_bass_kernel_spmd(nc, in_maps, core_ids=list(range(8)), trace=_trace)
    _CACHE["last_result"] = res
    full = np.zeros((B, S, 2 * HG), np.float32)
    for core in range(8):
        b, hg = core // 2, core % 2
        full[b, :, hg * HG:(hg + 1) * HG] = res.results[core]["out"]
    return full


# revision 9
# speedup vs baseline: 1.7510x; 1.1504x over previous
"""Distributed multi-head attention kernel for 8 TRN2 NeuronCores.

Sharding: core c handles batch b = c//2 and head-group hg = c%2 (4 of 8
heads = 256 output columns).  Output slices are disjoint -> no collectives;
the host concatenates the 8 slices.

Device algorithm (per core), bf16 matmuls / f32 softmax+finalize:
  - host permutes the key axis (unmasked keys first, ascending) and
    transposes inputs to [D, S]; only the first NU=ceil(max_unmasked/128)
    key chunks enter scores/exp/PV (sparse attention over v_mask) -- the
    remaining masked keys would contribute exactly exp(-1e10) = 0
  - scores in S^T layout [k', q]; the two heads of a pair use PE row
    groups 0-63 / 64-127 so their score matmuls run concurrently; one
    [128, 1024] PSUM tile holds both heads' scores for a q-tile and a
    single ACT exp (per-partition key bias; scale=0.125) covers both
  - causal masking: block-level skips from a liveness structure computed
    from v_mask (union over batches so the SPMD graph is identical on all
    cores); straddling blocks get per-core 0/1 masks multiplied in (bf16)
  - PV: O^T[65, q] accumulated in PSUM over key chunks; row 64 (ones
    column appended to VW) is the softmax denominator
  - dead queries (all causally-allowed keys masked; 0/0 in exact math but
    the reference's fp32 rounding yields a uniform average over
    singly-masked keys): host passes indicator columns in permuted order;
    tiny N=4 matmuls over all 16 chunks add the exact fix into output
    columns 0..3
  - finalize: PE-transpose O^T -> [q, 65] (bf16), scale by
    q_mask/rowsum, single 2MB output DMA
"""

import numpy as np
import ml_dtypes

BF = ml_dtypes.bfloat16
B, S, D = 4, 2048, 512
HG = 256          # output columns per core (4 heads x 64)
KS = 65           # head value width + ones column
NCH = 16          # total key chunks of 128
NEG = np.float32(-1e10)

_CACHE = {}


def _structure(v_mask):
    """Key permutations + block liveness (union over batches -> SPMD-safe)."""
    perms, n1s = [], []
    for b in range(B):
        unm = np.where(v_mask[b] == 1)[0]
        msk = np.where(v_mask[b] == 0)[0]
        perms.append(np.concatenate([unm, msk]))
        n1s.append(len(unm))
    NU = int(max(-(-n // 128) for n in n1s))
    live = set()
    band = set()
    for b in range(B):
        unm = perms[b][:n1s[b]]
        for c in range(NU):
            seg = unm[128 * c:min(128 * (c + 1), n1s[b])]
            if len(seg) == 0:
                continue
            lo, hi = int(seg[0]), int(seg[-1])
            for t in range(4):
                if lo > 512 * t + 511:
                    continue
                live.add((c, t))
                if hi > 512 * t:
                    band.add((c, t))
    live_lists = tuple(tuple(sorted(c for (c, tt) in live if tt == t))
                       for t in range(4))
    band_list = tuple(sorted(band))
    return perms, n1s, NU, live_lists, band_list


def _build(NU, live_lists, band_list):
    import concourse.bass as bass  # noqa: F401
    from concourse import bacc
    import concourse.mybir as mybir
    from concourse.tile import TileContext

    F32 = mybir.dt.float32
    BF16 = mybir.dt.bfloat16
    Exp = mybir.ActivationFunctionType.Exp
    nband = len(band_list)
    band_idx = {ct: i for i, ct in enumerate(band_list)}
    kp_tiles = -(-NU * 128 // 512)  # s-tiles of K to project

    nc = bacc.Bacc()
    qT = nc.declare_dram_parameter("qT", [D, S], BF16, isOutput=False)
    kT = nc.declare_dram_parameter("kT", [D, S], BF16, isOutput=False)
    vT = nc.declare_dram_parameter("vT", [D, S], BF16, isOutput=False)
    wq = nc.declare_dram_parameter("wq", [D, HG], BF16, isOutput=False)
    wk = nc.declare_dram_parameter("wk", [D, HG], BF16, isOutput=False)
    wv = nc.declare_dram_parameter("wv", [D, HG], BF16, isOutput=False)
    vbias = nc.declare_dram_parameter("vbias", [128, NCH], F32, isOutput=False)
    qmask = nc.declare_dram_parameter("qmask", [128, NCH], F32, isOutput=False)
    bmask = nc.declare_dram_parameter("bmask", [128, nband * 1024], BF16,
                                      isOutput=False)
    fixv = nc.declare_dram_parameter("fixv", [128, 4 * NCH], BF16, isOutput=False)
    ident = nc.declare_dram_parameter("ident", [128, 128], BF16, isOutput=False)
    ones4 = nc.declare_dram_parameter("ones4", [128, 4], BF16, isOutput=False)
    out = nc.declare_dram_parameter("out", [S, HG], F32, isOutput=True)

    with TileContext(nc) as tc:
        with tc.tile_pool(name="sb", bufs=1) as sb, \
             tc.tile_pool(name="ps", bufs=1, space="PSUM") as ps:

            def sbt(name, shape, dtype, bufs=1, tag=None):
                return sb.tile(shape, dtype, name=name, tag=tag or name, bufs=bufs)

            vbias_sb = sbt("vbias_sb", [128, NCH], F32)
            nc.sync.dma_start(out=vbias_sb, in_=vbias[:])
            qmask_sb = sbt("qmask_sb", [128, NCH], F32)
            nc.sync.dma_start(out=qmask_sb, in_=qmask[:])
            bmask_sb = sbt("bmask_sb", [128, nband * 1024], BF16)
            nc.sync.dma_start(out=bmask_sb, in_=bmask[:])
            fixv_sb = sbt("fixv_sb", [128, 4 * NCH], BF16)
            nc.sync.dma_start(out=fixv_sb, in_=fixv[:])
            ident_sb = sbt("ident_sb", [128, 128], BF16)
            nc.sync.dma_start(out=ident_sb, in_=ident[:])
            ones4_sb = sbt("ones4_sb", [128, 4], BF16)
            nc.sync.dma_start(out=ones4_sb, in_=ones4[:])

            w_sb = {}
            for nm, dram in (("q", wq), ("k", wk), ("v", wv)):
                for Dc in range(4):
                    t = sbt(f"w{nm}{Dc}", [128, HG], BF16)
                    nc.sync.dma_start(out=t, in_=dram[128 * Dc:128 * (Dc + 1), :])
                    w_sb[(nm, Dc)] = t

            qwT = [sbt(f"qwT{i}", [128, S], BF16) for i in range(2)]
            kwT = [sbt(f"kwT{i}", [128, S], BF16) for i in range(2)]
            vw = [sbt(f"vw{i}", [128, 4 * KS], BF16) for i in range(NCH)]

            def load_xT(dram):
                tiles = []
                for Dc in range(4):
                    t = sb.tile([128, S], BF16, name=f"xT{Dc}", tag=f"xT{Dc}",
                                bufs=2)
                    nc.sync.dma_start(out=t, in_=dram[128 * Dc:128 * (Dc + 1), :])
                    tiles.append(t)
                return tiles

            # ---- projections ----
            vt = load_xT(vT)
            for st in range(NCH):
                p = ps.tile([128, HG], F32, name="pprj", tag="psS", bufs=3)
                for Dc in range(4):
                    nc.tensor.matmul(p, vt[Dc][:, 128 * st:128 * (st + 1)],
                                     w_sb[("v", Dc)], start=(Dc == 0), stop=(Dc == 3))
                t = vw[st]
                nc.vector.tensor_copy(
                    t.rearrange("p (h j) -> p h j", j=KS)[:, :, 64:65],
                    ones4_sb.rearrange("p (h o) -> p h o", o=1))
                nc.vector.tensor_copy(
                    t.rearrange("p (h j) -> p h j", j=KS)[:, :, 0:64],
                    p.rearrange("p (h j) -> p h j", j=64))

            for src, dst, wnm, ntl in ((kT, kwT, "k", kp_tiles), (qT, qwT, "q", 4)):
                xt = load_xT(src)
                for dc in range(2):
                    for st2 in range(ntl):
                        p = ps.tile([128, 512], F32, name="pprj2", tag="psS", bufs=3)
                        for Dc in range(4):
                            nc.tensor.matmul(
                                p, w_sb[(wnm, Dc)][:, 128 * dc:128 * (dc + 1)],
                                xt[Dc][:, 512 * st2:512 * (st2 + 1)],
                                start=(Dc == 0), stop=(Dc == 3))
                        nc.scalar.copy(dst[dc][:, 512 * st2:512 * (st2 + 1)], p)

            # ---- attention: head pairs x q-tile passes, compacted keys ----
            ofin = sbt("ofin", [128, NCH * HG], F32)
            for dc in range(2):
                h0, h1 = 2 * dc, 2 * dc + 1
                kw_t, qw_t = kwT[dc], qwT[dc]
                for t in range(4):
                    lc = live_lists[t]
                    psO = {}
                    for hh in (h0, h1):
                        psO[hh] = ps.tile([KS, 512], F32, name=f"psO{hh}",
                                          tag="psO", bufs=2)
                    cend = NCH if t == 0 else lc[-1] + 1
                    for c in range(cend):
                        if c in lc:
                            psS = ps.tile([128, 1024], F32, name="psS",
                                          tag="psS", bufs=3)
                            for i, ho in enumerate((0, 64)):
                                nc.tensor.matmul(
                                    psS[:, 512 * i:512 * (i + 1)],
                                    kw_t[ho:ho + 64, 128 * c:128 * (c + 1)],
                                    qw_t[ho:ho + 64, 512 * t:512 * (t + 1)],
                                    start=True, stop=True)
                            U = sb.tile([128, 1024], BF16, name="U", tag="U",
                                        bufs=4)
                            nc.scalar.activation(U, psS, Exp,
                                                 bias=vbias_sb[:, c:c + 1],
                                                 scale=0.125)
                            if (c, t) in band_idx:
                                off = band_idx[(c, t)] * 1024
                                nc.vector.tensor_mul(
                                    U, U, bmask_sb[:, off:off + 1024])
                            for i, hh in enumerate((h0, h1)):
                                stop = (c == lc[-1]) if t > 0 else False
                                nc.tensor.matmul(psO[hh],
                                                 vw[c][:, KS * hh:KS * (hh + 1)],
                                                 U[:, 512 * i:512 * (i + 1)],
                                                 start=(c == lc[0]), stop=stop,
                                                 skip_group_check=True)
                        if t == 0:
                            for hh in (h0, h1):
                                nc.tensor.matmul(
                                    psO[hh][:, 0:4],
                                    vw[c][:, KS * hh:KS * (hh + 1)],
                                    fixv_sb[:, 4 * c:4 * (c + 1)],
                                    start=False, stop=(c == NCH - 1),
                                    skip_group_check=True)
                    # finalize this q-tile for both heads
                    for hh in (h0, h1):
                        ot = sb.tile([KS, 512], BF16, name="ot", tag="ot", bufs=2)
                        nc.vector.tensor_copy(ot, psO[hh])
                        tp = ps.tile([128, 4 * 66], BF16, name="tp", tag="psS",
                                     bufs=3)
                        for j in range(4):
                            nc.tensor.matmul(tp[:, 66 * j:66 * j + KS],
                                             ot[:, 128 * j:128 * (j + 1)],
                                             ident_sb[0:KS, 0:KS],
                                             is_transpose=True,
                                             start=(j == 0), stop=(j == 3),
                                             skip_group_check=True)
                        rs = sb.tile([128, 4], F32, name="rs", tag="rs", bufs=2)
                        nc.vector.tensor_scalar_add(
                            rs.rearrange("p (j o) -> p j o", o=1),
                            tp.rearrange("p (j f) -> p j f", f=66)[:, :, 64:65],
                            1e-30)
                        rcp = sb.tile([128, 4], F32, name="rcp", tag="rcp", bufs=2)
                        nc.vector.reciprocal(rcp, rs)
                        scl = sb.tile([128, 4], F32, name="scl", tag="scl", bufs=2)
                        nc.vector.tensor_mul(scl, rcp, qmask_sb[:, 4 * t:4 * (t + 1)])
                        for j in range(4):
                            col = (4 * t + j) * HG + 64 * hh
                            nc.vector.tensor_scalar_mul(
                                ofin[:, col:col + 64], tp[:, 66 * j:66 * j + 64],
                                scl[:, j:j + 1])
            nc.sync.dma_start(out=out.rearrange("(j p) n -> p j n", p=128),
                              in_=ofin.rearrange("p (j n) -> p j n", n=HG))

    nc.compile()
    return nc


def _prep_inputs(q, k, v, v_mask, q_mask, Wq, Wk, Wv, perms, n1s, band_list):
    q = np.asarray(q, np.float32)
    k = np.asarray(k, np.float32)
    v = np.asarray(v, np.float32)
    v_mask = np.asarray(v_mask, np.float32)
    q_mask = np.asarray(q_mask, np.float32)
    Wq = np.asarray(Wq, np.float32)
    Wk = np.asarray(Wk, np.float32)
    Wv = np.asarray(Wv, np.float32)
    ident = np.eye(128, dtype=np.float32)
    nband = len(band_list)

    in_maps = []
    for core in range(8):
        b, hg = core // 2, core % 2
        cs = slice(hg * HG, (hg + 1) * HG)
        perm, n1 = perms[b], n1s[b]
        vb = np.where(np.arange(S) < n1, np.float32(0), NEG).astype(np.float32)
        fix = np.zeros((S, 4), np.float32)
        if v_mask[b, 0] == 0:
            first_one = int(np.argmax(v_mask[b] > 0))
            ks_ = np.arange(S)
            for dj in range(min(first_one, 4)):
                sel = ((ks_ <= dj) & (v_mask[b] == 0)) | \
                      ((ks_ > dj) & (v_mask[b] == 1))
                fix[:, dj] = sel[perm].astype(np.float32)
        bm = np.zeros((128, nband * 1024), np.float32)
        for i, (c, t) in enumerate(band_list):
            kpos = perm[128 * c:128 * (c + 1)][:, None]
            m = (kpos <= (512 * t + np.arange(512))[None, :]).astype(np.float32)
            bm[:, 1024 * i:1024 * i + 512] = m
            bm[:, 1024 * i + 512:1024 * (i + 1)] = m
        in_maps.append({
            "qT": np.ascontiguousarray(q[b].T).astype(BF),
            "kT": np.ascontiguousarray(k[b][perm].T).astype(BF),
            "vT": np.ascontiguousarray(v[b][perm].T).astype(BF),
            "wq": np.ascontiguousarray(Wq[:, cs]).astype(BF),
            "wk": np.ascontiguousarray(Wk[:, cs]).astype(BF),
            "wv": np.ascontiguousarray(Wv[:, cs]).astype(BF),
            "vbias": np.ascontiguousarray(vb.reshape(NCH, 128).T),
            "qmask": np.ascontiguousarray(q_mask[b].reshape(NCH, 128).T),
            "bmask": bm.astype(BF),
            "fixv": np.ascontiguousarray(
                fix.reshape(NCH, 128, 4).transpose(1, 0, 2)
                .reshape(128, 4 * NCH)).astype(BF),
            "ident": ident.astype(BF),
            "ones4": np.ones((128, 4), BF),
        })
    return in_maps


def kernel(q, k, v, v_mask, q_mask, Wq, Wk, Wv, _trace=False):
    from concourse.bass_utils import run_bass_kernel_spmd

    v_mask_f = np.asarray(v_mask, np.float32)
    perms, n1s, NU, live_lists, band_list = _structure(v_mask_f)
    key = (NU, live_lists, band_list)
    if _CACHE.get("key") != key:
        _CACHE["nc"] = _build(NU, live_lists, band_list)
        _CACHE["key"] = key
    nc = _CACHE["nc"]
    in_maps = _prep_inputs(q, k, v, v_mask, q_mask, Wq, Wk, Wv,
                           perms, n1s, band_list)
    res = run_bass_kernel_spmd(nc, in_maps, core_ids=list(range(8)), trace=_trace)
    _CACHE["last_result"] = res
    full = np.zeros((B, S, 2 * HG), np.float32)
    for core in range(8):
        b, hg = core // 2, core % 2
        full[b, :, hg * HG:(hg + 1) * HG] = res.results[core]["out"]
    return full


# revision 10
# speedup vs baseline: 1.8168x; 1.0376x over previous
"""Distributed multi-head attention kernel for 8 TRN2 NeuronCores.

Sharding: core c handles batch b = c//2 and head-group hg = c%2 (4 of 8
heads = 256 output columns).  Output slices are disjoint -> no collectives;
the host concatenates the 8 slices.

Device algorithm (per core), bf16 matmuls / f32 softmax+finalize:
  - host permutes the key axis (unmasked keys first, ascending) and
    transposes inputs to [D, S]; only the first NU=ceil(max_unmasked/128)
    key chunks enter scores/exp/PV (sparse attention over v_mask) -- the
    remaining masked keys would contribute exactly exp(-1e10) = 0
  - scores in S^T layout [k', q]; the two heads of a pair use PE row
    groups 0-63 / 64-127 so their score matmuls run concurrently; one
    [128, 1024] PSUM tile holds both heads' scores for a q-tile and a
    single ACT exp (per-partition key bias; scale=0.125) covers both
  - causal masking: block-level skips from a liveness structure computed
    from v_mask (union over batches so the SPMD graph is identical on all
    cores); straddling blocks get per-core 0/1 masks multiplied in (bf16)
  - PV: O^T[65, q] accumulated in PSUM over key chunks; row 64 (ones
    column appended to VW) is the softmax denominator
  - dead queries (all causally-allowed keys masked; 0/0 in exact math but
    the reference's fp32 rounding yields a uniform average over
    singly-masked keys): host passes indicator columns in permuted order;
    tiny N=4 matmuls over all 16 chunks add the exact fix into output
    columns 0..3
  - finalize: PE-transpose O^T -> [q, 65] (bf16), scale by
    q_mask/rowsum, single 2MB output DMA
"""

import numpy as np
import ml_dtypes

BF = ml_dtypes.bfloat16
B, S, D = 4, 2048, 512
HG = 256          # output columns per core (4 heads x 64)
KS = 65           # head value width + ones column
NCH = 16          # total key chunks of 128
NEG = np.float32(-1e10)

_CACHE = {}


def _structure(v_mask):
    """Key permutations + block liveness (union over batches -> SPMD-safe)."""
    perms, n1s = [], []
    for b in range(B):
        unm = np.where(v_mask[b] == 1)[0]
        msk = np.where(v_mask[b] == 0)[0]
        perms.append(np.concatenate([unm, msk]))
        n1s.append(len(unm))
    NU = int(max(-(-n // 128) for n in n1s))
    live = set()
    band = set()
    for b in range(B):
        unm = perms[b][:n1s[b]]
        for c in range(NU):
            seg = unm[128 * c:min(128 * (c + 1), n1s[b])]
            if len(seg) == 0:
                continue
            lo, hi = int(seg[0]), int(seg[-1])
            for t in range(4):
                if lo > 512 * t + 511:
                    continue
                live.add((c, t))
                if hi > 512 * t:
                    band.add((c, t))
    live_lists = tuple(tuple(sorted(c for (c, tt) in live if tt == t))
                       for t in range(4))
    band_list = tuple(sorted(band))
    return perms, n1s, NU, live_lists, band_list


def _build(NU, live_lists, band_list):
    import concourse.bass as bass  # noqa: F401
    from concourse import bacc
    import concourse.mybir as mybir
    from concourse.tile import TileContext

    F32 = mybir.dt.float32
    BF16 = mybir.dt.bfloat16
    Exp = mybir.ActivationFunctionType.Exp
    nband = len(band_list)
    band_idx = {ct: i for i, ct in enumerate(band_list)}
    kp_tiles = -(-NU * 128 // 512)  # s-tiles of K to project

    nc = bacc.Bacc()
    qT = nc.declare_dram_parameter("qT", [D, S], BF16, isOutput=False)
    kT = nc.declare_dram_parameter("kT", [D, S], BF16, isOutput=False)
    vT = nc.declare_dram_parameter("vT", [D, S], BF16, isOutput=False)
    wq = nc.declare_dram_parameter("wq", [D, HG], BF16, isOutput=False)
    wk = nc.declare_dram_parameter("wk", [D, HG], BF16, isOutput=False)
    wv = nc.declare_dram_parameter("wv", [D, HG], BF16, isOutput=False)
    vbias = nc.declare_dram_parameter("vbias", [128, NCH], F32, isOutput=False)
    qmask = nc.declare_dram_parameter("qmask", [128, NCH], F32, isOutput=False)
    bmask = nc.declare_dram_parameter("bmask", [128, nband * 512], BF16,
                                      isOutput=False)
    fixv = nc.declare_dram_parameter("fixv", [128, 4 * NCH], BF16, isOutput=False)
    ident = nc.declare_dram_parameter("ident", [128, 128], BF16, isOutput=False)
    ones4 = nc.declare_dram_parameter("ones4", [128, 4], BF16, isOutput=False)
    out = nc.declare_dram_parameter("out", [S, HG], F32, isOutput=True)

    with TileContext(nc) as tc:
        with tc.tile_pool(name="sb", bufs=1) as sb, \
             tc.tile_pool(name="ps", bufs=1, space="PSUM") as ps:

            def sbt(name, shape, dtype, bufs=1, tag=None):
                return sb.tile(shape, dtype, name=name, tag=tag or name, bufs=bufs)

            vbias_sb = sbt("vbias_sb", [128, NCH], F32)
            nc.sync.dma_start(out=vbias_sb, in_=vbias[:])
            qmask_sb = sbt("qmask_sb", [128, NCH], F32)
            nc.sync.dma_start(out=qmask_sb, in_=qmask[:])
            bmask_sb = sbt("bmask_sb", [128, nband * 512], BF16)
            nc.sync.dma_start(out=bmask_sb, in_=bmask[:])
            fixv_sb = sbt("fixv_sb", [128, 4 * NCH], BF16)
            nc.sync.dma_start(out=fixv_sb, in_=fixv[:])
            ident_sb = sbt("ident_sb", [128, 128], BF16)
            nc.sync.dma_start(out=ident_sb, in_=ident[:])
            ones4_sb = sbt("ones4_sb", [128, 4], BF16)
            nc.sync.dma_start(out=ones4_sb, in_=ones4[:])

            w_sb = {}
            for nm, dram in (("q", wq), ("k", wk), ("v", wv)):
                for Dc in range(4):
                    t = sbt(f"w{nm}{Dc}", [128, HG], BF16)
                    nc.sync.dma_start(out=t, in_=dram[128 * Dc:128 * (Dc + 1), :])
                    w_sb[(nm, Dc)] = t

            qwT = [sbt(f"qwT{i}", [128, S], BF16) for i in range(2)]
            kwT = [sbt(f"kwT{i}", [128, S], BF16) for i in range(2)]
            vw = [sbt(f"vw{i}", [128, 4 * KS], BF16) for i in range(NCH)]

            def load_xT(dram, pfx):
                tiles = []
                for Dc in range(4):
                    t = sb.tile([128, S], BF16, name=f"{pfx}xT{Dc}",
                                tag=f"{pfx}xT{Dc}", bufs=1)
                    nc.sync.dma_start(out=t, in_=dram[128 * Dc:128 * (Dc + 1), :])
                    tiles.append(t)
                return tiles

            # ---- projections (all loads issued up front) ----
            vt = load_xT(vT, "v")
            kt = load_xT(kT, "k")
            qt = load_xT(qT, "q")
            for st in range(NCH):
                p = ps.tile([128, HG], F32, name="pprj", tag="psS", bufs=3)
                for Dc in range(4):
                    nc.tensor.matmul(p, vt[Dc][:, 128 * st:128 * (st + 1)],
                                     w_sb[("v", Dc)], start=(Dc == 0), stop=(Dc == 3))
                t = vw[st]
                nc.vector.tensor_copy(
                    t.rearrange("p (h j) -> p h j", j=KS)[:, :, 64:65],
                    ones4_sb.rearrange("p (h o) -> p h o", o=1))
                nc.vector.tensor_copy(
                    t.rearrange("p (h j) -> p h j", j=KS)[:, :, 0:64],
                    p.rearrange("p (h j) -> p h j", j=64))

            for dc in range(2):
                for xt, dst, wnm, ntl in ((kt, kwT, "k", kp_tiles),
                                          (qt, qwT, "q", 4)):
                    for st2 in range(ntl):
                        p = ps.tile([128, 512], F32, name="pprj2", tag="psS", bufs=3)
                        for Dc in range(4):
                            nc.tensor.matmul(
                                p, w_sb[(wnm, Dc)][:, 128 * dc:128 * (dc + 1)],
                                xt[Dc][:, 512 * st2:512 * (st2 + 1)],
                                start=(Dc == 0), stop=(Dc == 3))
                        nc.scalar.copy(dst[dc][:, 512 * st2:512 * (st2 + 1)], p)

            # ---- attention: head pairs x q-tile passes, compacted keys ----
            ofin = sbt("ofin", [128, NCH * HG], F32)
            for dc in range(2):
                h0, h1 = 2 * dc, 2 * dc + 1
                kw_t, qw_t = kwT[dc], qwT[dc]
                for t in range(4):
                    lc = live_lists[t]
                    psO = {}
                    for hh in (h0, h1):
                        psO[hh] = ps.tile([KS, 512], F32, name=f"psO{hh}",
                                          tag="psO", bufs=2)
                    cend = NCH if t == 0 else lc[-1] + 1
                    for c in range(cend):
                        if c in lc:
                            psS = ps.tile([128, 1024], F32, name="psS",
                                          tag="psS", bufs=3)
                            for i, ho in enumerate((0, 64)):
                                nc.tensor.matmul(
                                    psS[:, 512 * i:512 * (i + 1)],
                                    kw_t[ho:ho + 64, 128 * c:128 * (c + 1)],
                                    qw_t[ho:ho + 64, 512 * t:512 * (t + 1)],
                                    start=True, stop=True)
                            U = sb.tile([128, 1024], BF16, name="U", tag="U",
                                        bufs=6)
                            for i, hh in enumerate((h0, h1)):
                                Ui = U[:, 512 * i:512 * (i + 1)]
                                nc.scalar.activation(Ui, psS[:, 512 * i:512 * (i + 1)],
                                                     Exp,
                                                     bias=vbias_sb[:, c:c + 1],
                                                     scale=0.125)
                                if (c, t) in band_idx:
                                    off = band_idx[(c, t)] * 512
                                    nc.vector.tensor_mul(
                                        Ui, Ui, bmask_sb[:, off:off + 512])
                                stop = (c == lc[-1]) if t > 0 else False
                                nc.tensor.matmul(psO[hh],
                                                 vw[c][:, KS * hh:KS * (hh + 1)],
                                                 Ui,
                                                 start=(c == lc[0]), stop=stop,
                                                 skip_group_check=True)
                        if t == 0:
                            for hh in (h0, h1):
                                nc.tensor.matmul(
                                    psO[hh][:, 0:4],
                                    vw[c][:, KS * hh:KS * (hh + 1)],
                                    fixv_sb[:, 4 * c:4 * (c + 1)],
                                    start=False, stop=(c == NCH - 1),
                                    skip_group_check=True)
                    # finalize this q-tile for both heads
                    for hh in (h0, h1):
                        ot = sb.tile([KS, 512], BF16, name="ot", tag="ot", bufs=2)
                        nc.vector.tensor_copy(ot, psO[hh])
                        tp = ps.tile([128, 4 * 66], BF16, name="tp", tag="psS",
                                     bufs=3)
                        for j in range(4):
                            nc.tensor.matmul(tp[:, 66 * j:66 * j + KS],
                                             ot[:, 128 * j:128 * (j + 1)],
                                             ident_sb[0:KS, 0:KS],
                                             is_transpose=True,
                                             start=(j == 0), stop=(j == 3),
                                             skip_group_check=True)
                        rs = sb.tile([128, 4], F32, name="rs", tag="rs", bufs=2)
                        nc.vector.tensor_scalar_add(
                            rs.rearrange("p (j o) -> p j o", o=1),
                            tp.rearrange("p (j f) -> p j f", f=66)[:, :, 64:65],
                            1e-30)
                        rcp = sb.tile([128, 4], F32, name="rcp", tag="rcp", bufs=2)
                        nc.vector.reciprocal(rcp, rs)
                        scl = sb.tile([128, 4], F32, name="scl", tag="scl", bufs=2)
                        nc.vector.tensor_mul(scl, rcp, qmask_sb[:, 4 * t:4 * (t + 1)])
                        for j in range(4):
                            col = (4 * t + j) * HG + 64 * hh
                            nc.vector.tensor_scalar_mul(
                                ofin[:, col:col + 64], tp[:, 66 * j:66 * j + 64],
                                scl[:, j:j + 1])
            nc.sync.dma_start(out=out.rearrange("(j p) n -> p j n", p=128),
                              in_=ofin.rearrange("p (j n) -> p j n", n=HG))

    nc.compile()
    return nc


def _prep_inputs(q, k, v, v_mask, q_mask, Wq, Wk, Wv, perms, n1s, band_list):
    q = np.asarray(q, np.float32)
    k = np.asarray(k, np.float32)
    v = np.asarray(v, np.float32)
    v_mask = np.asarray(v_mask, np.float32)
    q_mask = np.asarray(q_mask, np.float32)
    Wq = np.asarray(Wq, np.float32)
    Wk = np.asarray(Wk, np.float32)
    Wv = np.asarray(Wv, np.float32)
    ident = np.eye(128, dtype=np.float32)
    nband = len(band_list)

    in_maps = []
    for core in range(8):
        b, hg = core // 2, core % 2
        cs = slice(hg * HG, (hg + 1) * HG)
        perm, n1 = perms[b], n1s[b]
        vb = np.where(np.arange(S) < n1, np.float32(0), NEG).astype(np.float32)
        fix = np.zeros((S, 4), np.float32)
        if v_mask[b, 0] == 0:
            first_one = int(np.argmax(v_mask[b] > 0))
            ks_ = np.arange(S)
            for dj in range(min(first_one, 4)):
                sel = ((ks_ <= dj) & (v_mask[b] == 0)) | \
                      ((ks_ > dj) & (v_mask[b] == 1))
                fix[:, dj] = sel[perm].astype(np.float32)
        bm = np.zeros((128, nband * 512), np.float32)
        for i, (c, t) in enumerate(band_list):
            kpos = perm[128 * c:128 * (c + 1)][:, None]
            bm[:, 512 * i:512 * (i + 1)] = (
                kpos <= (512 * t + np.arange(512))[None, :]).astype(np.float32)
        in_maps.append({
            "qT": np.ascontiguousarray(q[b].T).astype(BF),
            "kT": np.ascontiguousarray(k[b][perm].T).astype(BF),
            "vT": np.ascontiguousarray(v[b][perm].T).astype(BF),
            "wq": np.ascontiguousarray(Wq[:, cs]).astype(BF),
            "wk": np.ascontiguousarray(Wk[:, cs]).astype(BF),
            "wv": np.ascontiguousarray(Wv[:, cs]).astype(BF),
            "vbias": np.ascontiguousarray(vb.reshape(NCH, 128).T),
            "qmask": np.ascontiguousarray(q_mask[b].reshape(NCH, 128).T),
            "bmask": bm.astype(BF),
            "fixv": np.ascontiguousarray(
                fix.reshape(NCH, 128, 4).transpose(1, 0, 2)
                .reshape(128, 4 * NCH)).astype(BF),
            "ident": ident.astype(BF),
            "ones4": np.ones((128, 4), BF),
        })
    return in_maps


def kernel(q, k, v, v_mask, q_mask, Wq, Wk, Wv, _trace=False):
    from concourse.bass_utils import run_bass_kernel_spmd

    v_mask_f = np.asarray(v_mask, np.float32)
    perms, n1s, NU, live_lists, band_list = _structure(v_mask_f)
    key = (NU, live_lists, band_list)
    if _CACHE.get("key") != key:
        _CACHE["nc"] = _build(NU, live_lists, band_list)
        _CACHE["key"] = key
    nc = _CACHE["nc"]
    in_maps = _prep_inputs(q, k, v, v_mask, q_mask, Wq, Wk, Wv,
                           perms, n1s, band_list)
    res = run_bass_kernel_spmd(nc, in_maps, core_ids=list(range(8)), trace=_trace)
    _CACHE["last_result"] = res
    full = np.zeros((B, S, 2 * HG), np.float32)
    for core in range(8):
        b, hg = core // 2, core % 2
        full[b, :, hg * HG:(hg + 1) * HG] = res.results[core]["out"]
    return full
